# revision 18
# baseline (speedup 1.0000x reference)
"""Trainium2 Bass kernel for nn_ActorSpine (population-coding encoder MLP actor).

Reference computation (per sample):
  spine = sigmoid((state[:, :, None] - mean_enc) / std_enc)  # [B, 128, 10]
  a1 = relu(spine.reshape(B, 1280) @ W1.T + b1)              # [B, 2048]
  a2 = relu(a1 @ W2.T + b2)                                  # [B, 2048]
  a3 = a2 @ W3.T + b3                                        # [B, 320]
  raw = einsum('bak,ak->ba', a3.reshape(B, 32, 10), Wd[:, 0]) + bd
  out = tanh(raw)                                            # [B, 32]

Strategy: pure data parallel over 8 cores (2048 samples each).
Host-side folding:
  - decoder conv folds into W3: W3p[a, h] = sum_k Wd[a,0,k] * W3[a*10+k, h],
    b3p[a] = sum_k Wd[a,0,k]*b3[a*10+k] + bd[a]  -> final layer is [32, 2048]
  - encoder contraction index permuted j' = k*128 + d so spine k-tiles are
    plain per-partition sigmoid activations of stateT; W1 columns permuted to
    match.
Device: activations kept transposed [feature, batch]. Layers 1+2 run in
fp8-e4m3 with perf_mode=DoubleRow (two k-subtiles per matmul -> 2x the
fp16 column rate, ~215ns per [128x256]x[256x512] mm); fp32 PSUM. Power-of-2
scales fold away: W1 host-scaled by 32 so h1 = relu(psum + 32*b1) is 32*a1,
quantized straight to fp8 by the DVE relu (relu commutes with positive
scale); W2 host-scaled by 64 so h2 (fp16) carries scale 2048; W3p
host-scaled by 2048/64 so the final tanh uses a 1/64 input scale constant.
End-to-end rel err ~9e-3 vs the 2e-2 gate (bias terms dominate the output
norm, diluting fp8 quantization noise on the data path).

Schedule (from perfetto-trace iteration):
  - weights live in DRAM pre-transposed with the SBUF partition dim
    outermost, so each weight matrix arrives in 2-3 large DMAs (a small
    transfer pays ~0.5us fixed overhead; singleton per-tile DMAs starved
    sweep-0 L1);
  - sweep-0 sigmoids are emitted BEFORE the weight dma_starts: the sync
    engine batches DMA-completion waits by program order, and emitting them
    after made the first sigmoid wait on five weight-block DMAs (~3.5us);
  - sigmoids for chunk n+1 are emitted during sweep n (ScalarE is in-order:
    they must sit AHEAD of tanh(n), which waits on the PE sel-matmul --
    otherwise every sweep seam idles PE ~2us and the idle re-throttles the
    PE clock gate to half rate for another 3.4us);
  - the layer-3 tail of sweep n (last quad, psum->fp16 copy, selection
    matmul, tanh, out-DMA) is deferred into the first two L1 groups of
    sweep n+1 so the DVE copy never blocks PE;
  - earlier L3 quads run inline, lagged two m-tiles behind their h2 relus;
  - PE warmup matmuls on an uninitialized tile (no producers -> start right
    after the engine preamble) open the HAM clock gate during the DMA
    window; a separate 1-element dummy activation preloads the ScalarE
    act table without making the warmup depend on ScalarE.
"""

import numpy as np
import ml_dtypes

import concourse.mybir as mybir
import concourse.tile as tile
from concourse import bacc
from concourse.bass_utils import run_bass_kernel_spmd

# Problem dims (hardcoded per harness contract)
B = 16384
D = 128
ENC_K = 10
ACT_DIM = 32
DEC_K = 10
H0 = 2048
H1 = 2048
NCORES = 8
BL = B // NCORES  # 2048 samples per core
NT = 512          # moving-dim tile (one PSUM bank of fp32)
NSUB = BL // NT   # 4
M1 = H0 // 128    # 16 m-tiles for layer 1
K1 = ENC_K        # 10 k-tiles for layer 1 (permuted encoder)
Q1 = K1 // 2      # 5 DoubleRow k-pairs
M2 = H1 // 128    # 16
K2 = H0 // 128    # 16
Q2 = K2 // 2      # 8 DoubleRow k-pairs
K3 = H1 // 128    # 16

F8 = mybir.dt.float8e4
F16 = mybir.dt.float16
F32 = mybir.dt.float32

# fp8 scale plumbing (all powers of 2; folded host-side into weights/biases)
SW1 = 32.0            # W1 and h1 fp8 scale
SW2 = 64.0            # W2 fp8 scale
SH2 = SW1 * SW2       # h2 (fp16) scale = 2048
SW3 = 32.0            # W3p divided by this -> psum3 scale SH2/SW3 = 64
TANH_SCALE = SW3 / SH2  # 1/64 input scale on the final tanh

DR = mybir.MatmulPerfMode.DoubleRow

_cached = {}


def _build_program():
    if "nc" in _cached:
        return _cached["nc"]

    nc = bacc.Bacc("TRN2", target_bir_lowering=False, debug=False,
                   num_devices=NCORES)

    stateT = nc.dram_tensor("stateT", [D, BL], F32, kind="ExternalInput")
    # weight layouts: [j, m, k, p]; j = within-tile contraction (SBUF
    # partition dim, outermost so one DMA descriptor per partition),
    # k = k-tile, p = output partition (lhsT free dim)
    w1t = nc.dram_tensor("w1t", [128, M1, K1, 128], F8, kind="ExternalInput")
    w2t = nc.dram_tensor("w2t", [128, M2, K2, 128], F8, kind="ExternalInput")
    w3t = nc.dram_tensor("w3t", [128, K3, ACT_DIM], F16, kind="ExternalInput")
    # scalars layout (per partition p): [0:10] enc_scale, [10:20] enc_bias,
    # [20:36] 32*b1, [36:52] 2048*b2, [52] b3p (partitions 0..31)
    scal = nc.dram_tensor("scal", [128, 53], F32, kind="ExternalInput")
    selt = nc.dram_tensor("selt", [128, ACT_DIM], F16, kind="ExternalInput")
    out = nc.dram_tensor("out", [ACT_DIM, BL], F32, kind="ExternalOutput")

    ADD = mybir.AluOpType.add
    MAX = mybir.AluOpType.max

    with tile.TileContext(nc) as tc:
        with (
            tc.tile_pool(name="consts", bufs=1) as consts,
            tc.tile_pool(name="acts", bufs=1) as acts,
            tc.tile_pool(name="h2p", bufs=18) as h2p,
            tc.tile_pool(name="w1p", bufs=1) as w1p,
            tc.tile_pool(name="w2p", bufs=1) as w2p,
            tc.tile_pool(name="outp", bufs=2) as outp,
            tc.tile_pool(name="psum", bufs=1, space="PSUM") as psum_pool,
        ):
            # state chunk 0 + scalars on the ACT DGE queue, separate from
            # the weight transfers: the scheduler coalesces DMA-completion
            # waits per queue, and sharing a queue with W1 made the first
            # sigmoid wait for megabytes of weights. A tiny GPSIMD read
            # after `sc` forces an exact sync boundary.
            st = acts.tile([D, BL], F32, tag="state")
            nc.scalar.dma_start(out=st[:, 0:NT], in_=stateT[:, 0:NT])
            sc = consts.tile([128, 53], F32)
            nc.scalar.dma_start(out=sc, in_=scal[:, :])
            sync_t = consts.tile([1, 8], F32, tag="dmasync")
            nc.gpsimd.tensor_copy(sync_t[0:1, 0:1], sc[0:1, 0:1])

            # act-table trigger: tiny Sigmoid+Tanh on a dummy tile preloads
            # the ScalarE activation table during the preamble, without
            # making anything else depend on it
            dum = consts.tile([1, 4], F32, tag="dummy")
            nc.scalar.activation(dum[0:1, 0:1], dum[0:1, 0:1],
                                 mybir.ActivationFunctionType.Sigmoid)
            nc.scalar.activation(dum[0:1, 1:2], dum[0:1, 1:2],
                                 mybir.ActivationFunctionType.Tanh)
            nc.scalar.activation(dum[0:1, 2:3], dum[0:1, 2:3],
                                 mybir.ActivationFunctionType.Relu)

            w1all = w1p.tile([128, M1, K1, 128], F8, tag="w1")
            w2all = w2p.tile([128, M2, K2, 128], F8, tag="w2")
            w3sb = consts.tile([128, K3, ACT_DIM], F16, tag="w3")
            sel_sb = consts.tile([128, ACT_DIM], F16, tag="sel")

            # Persistent PSUM accumulators, rotated manually. Banks are
            # zeroed by DVE several groups before reuse, and matmul groups
            # run WITHOUT start=True: the group-start bank-clear blocks the
            # LDWEIGHTS pull-ahead and costs ~100ns per group.
            NPS = 5
            ps_tiles = [psum_pool.tile([128, NT], F32, tag=f"ps{i}",
                                       name=f"ps{i}")
                        for i in range(NPS)]
            ps_idx = [0]

            def next_ps():
                t = ps_tiles[ps_idx[0] % NPS]
                ps_idx[0] += 1
                return t

            # layer-3 col-packed accumulators (full bank, 4 col-groups) and
            # the [32, NT] reduce target
            p3_tiles = [psum_pool.tile([128, NT], F32, tag=f"p3_{i}",
                                       name=f"p3t_{i}")
                        for i in range(2)]
            psr = psum_pool.tile([ACT_DIM, NT], F32, tag="psr", name="psr")

            # ---- PE warmup: dummy matmuls on an uninitialized tile (no
            # producers, so they dispatch the moment the preamble ends) keep
            # PE busy through the DMA window and open the HAM clock gate.
            # Results are garbage and discarded (psr is DVE-zeroed below).
            wz = consts.tile([128, NT], F16, tag="warmzero")
            # 1-element DVE write so the tile gets allocated; the rest stays
            # uninitialized garbage (results are discarded)
            nc.vector.memset(wz[0:1, 0:1], 0.0)
            NWARM = 6
            for w in range(NWARM):
                nc.tensor.matmul(
                    psr, wz[:, :ACT_DIM], wz,
                    start=(w == 0), stop=(w == NWARM - 1),
                    skip_group_check=True)

            # k-PAIR tiles (DoubleRow consumes two adjacent k-subtiles per
            # matmul); per-pair tiles keep dependency tracking fine-grained
            spine = [acts.tile([128, 2, BL], F8, tag=f"spine{q}",
                               name=f"spine{q}")
                     for q in range(Q1)]
            h1 = [acts.tile([128, 2, BL], F8, tag=f"h1_{p}", name=f"h1_{p}")
                  for p in range(Q2)]

            def emit_sig(n):
                ns = slice(n * NT, (n + 1) * NT)
                for k in range(K1):
                    nc.scalar.activation(
                        spine[k // 2][:, k % 2, ns], st[:, ns],
                        mybir.ActivationFunctionType.Sigmoid,
                        bias=sc[:, 10 + k:11 + k], scale=sc[:, k:k + 1])

            # sweep-0 sigmoids at scheduler priority 0: their sync-engine
            # wait ops must sit at the SP queue head, before anything tied
            # to the weight DMAs, so they gate on the state-chunk-0 + scal
            # transfers only (measured otherwise: first sigmoid at 18us,
            # PE idle 11us). This also pushes the W2 triggers behind the
            # sigmoids on ScalarE, giving W1 the full HBM bandwidth first.
            with tc.high_priority():
                emit_sig(0)

            for t in ps_tiles + p3_tiles:
                nc.vector.memset(t, 0.0)

            # Startup-critical W1 chunks + small tensors on the SP hardware
            # DGE queue; W2 bulk on the second (Activation) queue. The ACT
            # triggers sit AFTER the sigmoids in the ScalarE stream, which
            # (a) keeps the sigmoids at the queue head and (b) delays the
            # W2 transfers so W1 gets the full HBM bandwidth first (running
            # both queues at once starved the critical W1a chunk). Only two
            # ACT triggers are issued, below the DGE outstanding limit, so
            # they never block later ScalarE work.
            nc.sync.dma_start(out=w1all[:, 0:NPS], in_=w1t[:, 0:NPS])
            nc.gpsimd.tensor_copy(sync_t[0:1, 1:2],
                                  w1all[0:1, 0, 0, 0:1].bitcast(mybir.dt.uint8))
            nc.sync.dma_start(out=w1all[:, NPS:10], in_=w1t[:, NPS:10])
            nc.gpsimd.tensor_copy(sync_t[0:1, 2:3],
                                  w1all[0:1, NPS, 0, 0:1].bitcast(mybir.dt.uint8))
            nc.sync.dma_start(out=w1all[:, 10:M1], in_=w1t[:, 10:M1])
            nc.sync.dma_start(out=st[:, NT:BL], in_=stateT[:, NT:BL])
            nc.sync.dma_start(out=w3sb, in_=w3t[:, :, :])
            nc.sync.dma_start(out=sel_sb, in_=selt[:, :])
            nc.scalar.dma_start(out=w2all[:, 0:8], in_=w2t[:, 0:8])
            nc.gpsimd.tensor_copy(sync_t[0:1, 3:4],
                                  w2all[0:1, 0, 0, 0:1].bitcast(mybir.dt.uint8))
            nc.scalar.dma_start(out=w2all[:, 8:M2], in_=w2t[:, 8:M2])

            def emit_quad(q, p3, h2s):
                # one col-packed quad: 4 concurrent 32-wide matmuls.
                # q0 starts the accumulation group (clears the bank), so no
                # DVE pre-zeroing of p3 is ever needed. Priority 0 makes the
                # scheduler pop all 4 adjacently the moment their h2 inputs
                # are ready — scattered between DR matmuls they each pay two
                # un-hidden LDWEIGHTS switches and lose the col-group
                # concurrency (~1us per quad, measured).
                with tc.high_priority():
                    for j in range(4):
                        nc.tensor.matmul(
                            p3[32 * j:32 * (j + 1), :], w3sb[:, 4 * q + j, :],
                            h2s[j], start=(q == 0), stop=False,
                            skip_group_check=True, tile_position=(0, 32 * j))

            def emit_fin(n, p3):
                # cross-col-group reduce + tanh + store for sweep n. The s3
                # copy runs on ScalarE (not DVE) and the sel matmul is its
                # own start/stop group (psr needs no zeroing): the whole
                # tail touches only ScalarE+PE+DMA, so it can never wedge
                # the DVE relu/memset stream that feeds the PSUM rotation.
                s3 = outp.tile([128, NT], F16, tag="s3", name=f"s3_{n}")
                nc.scalar.activation(s3, p3,
                                     mybir.ActivationFunctionType.Copy)
                with tc.high_priority():
                    nc.tensor.matmul(psr, sel_sb, s3, start=True,
                                     stop=True, skip_group_check=True)
                ot = outp.tile([ACT_DIM, NT], F32, tag="ot", name=f"ot_{n}")
                nc.scalar.activation(
                    ot, psr, mybir.ActivationFunctionType.Tanh,
                    bias=sc[:ACT_DIM, 52:53], scale=TANH_SCALE)
                nc.sync.dma_start(out=out[:, n * NT:(n + 1) * NT], in_=ot)

            def l1_mms(ps, m, ns):
                for q in range(Q1):
                    nc.tensor.matmul(
                        ps, w1all[:, m, 2 * q:2 * q + 2, :],
                        spine[q][:, :, ns],
                        start=False, stop=False, skip_group_check=True,
                        perf_mode=DR)

            # deferred layer-3 tail of the previous sweep: (kind, args)
            tail = []

            # ---- fully interleaved per-column-chunk sweeps:
            # sigmoid(n+1) -> L1 m-sweep(n) [+ prev sweep's L3 tail]
            # -> L2 m-sweep(n) [+ inline lagged quads]
            for n in range(NSUB):
                ns = slice(n * NT, (n + 1) * NT)

                m_start = 0
                if n == 0:
                    # k-pair-striped cohort over all 5 banks: each pair-wave
                    # only needs two freshly produced spine slices, so real
                    # L1 work runs during the ScalarE sigmoid ramp.
                    m_start = NPS
                    cohort = [next_ps() for _ in range(NPS)]
                    for q in range(Q1):
                        for m in range(NPS):
                            nc.tensor.matmul(
                                cohort[m], w1all[:, m, 2 * q:2 * q + 2, :],
                                spine[q][:, :, ns],
                                start=False, stop=False,
                                skip_group_check=True, perf_mode=DR)
                    for m in range(NPS):
                        nc.vector.tensor_scalar(
                            h1[m // 2][:, m % 2, ns], cohort[m],
                            sc[:, 20 + m:21 + m], 0.0, ADD, MAX)
                        nc.vector.memset(cohort[m], 0.0)

                for idx, m in enumerate(range(m_start, M1)):
                    if n == 0 and m == m_start:
                        # p3_tiles[1] is idle until sweep 1's layer 3 — use
                        # it here so this group does not wait on the cohort's
                        # first relu+memset to release a rotation bank.
                        ps = p3_tiles[1]
                    else:
                        ps = next_ps()
                    l1_mms(ps, m, ns)
                    # relus alternate DVE/ScalarE: DVE alone (relu+memset
                    # ~1.23us) cannot keep up with the 1.08us L1 PE groups,
                    # and the rotation memsets must never starve
                    if m % 2 == 0:
                        nc.vector.tensor_scalar(
                            h1[m // 2][:, m % 2, ns], ps,
                            sc[:, 20 + m:21 + m], 0.0, ADD, MAX)
                    else:
                        nc.scalar.activation(
                            h1[m // 2][:, m % 2, ns], ps,
                            mybir.ActivationFunctionType.Relu,
                            bias=sc[:, 20 + m:21 + m])
                    nc.vector.memset(ps, 0.0)
                    # previous sweep's L3 tail, spread over the first two
                    # L1 groups (PE reaches the quad after ~1.1us of L1, by
                    # which time the last h2 relu is done; the sel matmul
                    # lands after the DVE s3 copy)
                    while tail and tail[0][0] <= idx:
                        _, fn, args = tail.pop(0)
                        fn(*args)

                if n + 1 < NSUB:
                    emit_sig(n + 1)

                p3 = p3_tiles[n % 2]
                h2s_sweep = []
                for m in range(M2):
                    ps = next_ps()
                    for q in range(Q2):
                        nc.tensor.matmul(
                            ps, w2all[:, m, 2 * q:2 * q + 2, :],
                            h1[q][:, :, ns],
                            start=False, stop=False, skip_group_check=True,
                            perf_mode=DR)
                    h2m = h2p.tile([128, NT], F16, tag="h2")
                    nc.vector.tensor_scalar(
                        h2m, ps, sc[:, 36 + m:37 + m], 0.0, ADD, MAX)
                    nc.vector.memset(ps, 0.0)
                    h2s_sweep.append(h2m)
                    # quads q0-q2 inline, lagged two m-tiles behind their
                    # relus; q3 is deferred into the next sweep's L1
                    if m in (5, 9, 13):
                        emit_quad((m - 5) // 4, p3, h2s_sweep[m - 5:m - 1])

                if n + 1 < NSUB:
                    tail = [(0, emit_quad, (3, p3, h2s_sweep[12:16])),
                            (1, emit_fin, (n, p3))]
                else:
                    emit_quad(3, p3, h2s_sweep[12:16])
                    emit_fin(n, p3)

    nc.compile()
    _cached["nc"] = nc
    return nc


def _q8(x):
    """fp32 -> TRN e4m3 bytes (clip to +-240; bit-identical to OCP there)."""
    return np.clip(x, -240.0, 240.0).astype(ml_dtypes.float8_e4m3fn)


def _prep_inputs(state, mean_enc, std_enc, W1, b1, W2, b2, W3, b3, Wd, bd):
    f32 = np.float32
    state = np.asarray(state, f32)
    mean_enc = np.asarray(mean_enc, f32)
    std_enc = np.asarray(std_enc, f32)
    W1 = np.asarray(W1, f32)
    b1 = np.asarray(b1, f32)
    W2 = np.asarray(W2, f32)
    b2 = np.asarray(b2, f32)
    W3 = np.asarray(W3, f32)
    b3 = np.asarray(b3, f32)
    Wd = np.asarray(Wd, f32)
    bd = np.asarray(bd, f32)

    # Fold decoder grouped conv into layer 3
    wd = Wd[:, 0, :]                                   # [32, 10]
    W3p = np.einsum("ak,akh->ah", wd, W3.reshape(ACT_DIM, DEC_K, H1))
    b3p = (b3.reshape(ACT_DIM, DEC_K) * wd).sum(1) + bd  # [32]

    # Permute encoder contraction: j' = k*128 + d
    W1p = W1.reshape(H0, D, ENC_K).transpose(0, 2, 1).reshape(H0, D * ENC_K)

    # Pre-tiled weight layouts: [j, m, k, p] (partition dim outermost so
    # each weight matrix is a handful of large contiguous DMAs)
    w1t = np.ascontiguousarray(
        _q8((W1p * SW1).reshape(M1, 128, K1, 128).transpose(3, 0, 2, 1)))
    w2t = np.ascontiguousarray(
        _q8((W2 * SW2).reshape(M2, 128, K2, 128).transpose(3, 0, 2, 1)))
    w3t = np.ascontiguousarray(
        (W3p / SW3).reshape(ACT_DIM, K3, 128).transpose(2, 1, 0)
        .astype(np.float16))

    scal = np.zeros((128, 53), f32)
    scal[:, 0:10] = 1.0 / std_enc[0]                   # enc scale [128, 10]
    scal[:, 10:20] = -mean_enc[0] / std_enc[0]         # enc bias
    scal[:, 20:36] = SW1 * b1.reshape(M1, 128).T
    scal[:, 36:52] = SH2 * b2.reshape(M2, 128).T
    scal[:ACT_DIM, 52] = b3p
    scal = np.ascontiguousarray(scal)

    sel = np.tile(np.eye(ACT_DIM, dtype=np.float16), (4, 1))

    in_maps = []
    for c in range(NCORES):
        shard = np.ascontiguousarray(state[c * BL:(c + 1) * BL].T)  # [128, BL]
        in_maps.append({
            "stateT": shard, "w1t": w1t, "w2t": w2t, "w3t": w3t,
            "scal": scal, "selt": sel,
        })
    return in_maps


def kernel(**inputs):
    nc = _build_program()
    in_maps = _prep_inputs(**inputs)
    res = run_bass_kernel_spmd(nc, in_maps, core_ids=list(range(NCORES)))
    out = np.concatenate(
        [res.results[c]["out"].T for c in range(NCORES)], axis=0)
    return np.ascontiguousarray(out.astype(np.float32))


if __name__ == "__main__":
    rng = np.random.default_rng(0)
    state = rng.standard_normal((B, D), dtype=np.float32)
    mean = np.broadcast_to(
        np.linspace(-3, 3, ENC_K, dtype=np.float32), (1, D, ENC_K)).copy()
    std = np.full((1, D, ENC_K), 0.3872983346207417, np.float32)

    def lin(fan_in, fan_out):
        bound = 1 / np.sqrt(fan_in)
        return (rng.uniform(-bound, bound, (fan_out, fan_in)).astype(np.float32),
                rng.uniform(-bound, bound, fan_out).astype(np.float32))

    W1, b1 = lin(D * ENC_K, H0)
    W2, b2 = lin(H0, H1)
    W3, b3 = lin(H1, ACT_DIM * DEC_K)
    Wd = rng.uniform(-0.3, 0.3, (ACT_DIM, 1, DEC_K)).astype(np.float32)
    bd = rng.uniform(-0.3, 0.3, ACT_DIM).astype(np.float32)

    outp = kernel(state=state, mean_enc=mean, std_enc=std, W1=W1, b1=b1,
                  W2=W2, b2=b2, W3=W3, b3=b3, Wd=Wd, bd=bd)

    # numpy reference
    spine = 1 / (1 + np.exp(-(state[:, :, None] - mean) / std))
    a = np.maximum(spine.reshape(B, -1) @ W1.T + b1, 0)
    a = np.maximum(a @ W2.T + b2, 0)
    a = a @ W3.T + b3
    raw = np.einsum("bak,ak->ba", a.reshape(B, ACT_DIM, DEC_K), Wd[:, 0]) + bd
    ref = np.tanh(raw)
    rel = np.linalg.norm(outp - ref) / np.linalg.norm(ref)
    print("rel err:", rel, "max abs diff:", np.abs(outp - ref).max())


# revision 24
# speedup vs baseline: 1.0078x; 1.0078x over previous
"""Trainium2 Bass kernel for nn_ActorSpine (population-coding encoder MLP actor).

Reference computation (per sample):
  spine = sigmoid((state[:, :, None] - mean_enc) / std_enc)  # [B, 128, 10]
  a1 = relu(spine.reshape(B, 1280) @ W1.T + b1)              # [B, 2048]
  a2 = relu(a1 @ W2.T + b2)                                  # [B, 2048]
  a3 = a2 @ W3.T + b3                                        # [B, 320]
  raw = einsum('bak,ak->ba', a3.reshape(B, 32, 10), Wd[:, 0]) + bd
  out = tanh(raw)                                            # [B, 32]

Strategy: pure data parallel over 8 cores (2048 samples each).
Host-side folding:
  - decoder conv folds into W3: W3p[a, h] = sum_k Wd[a,0,k] * W3[a*10+k, h],
    b3p[a] = sum_k Wd[a,0,k]*b3[a*10+k] + bd[a]  -> final layer is [32, 2048]
  - encoder contraction index permuted j' = k*128 + d so spine k-tiles are
    plain per-partition sigmoid activations of stateT; W1 columns permuted to
    match.
Device: activations kept transposed [feature, batch]. Layers 1+2 run in
fp8-e4m3 with perf_mode=DoubleRow (two k-subtiles per matmul -> 2x the
fp16 column rate, ~215ns per [128x256]x[256x512] mm); fp32 PSUM. Power-of-2
scales fold away: W1 host-scaled by 32 so h1 = relu(psum + 32*b1) is 32*a1,
quantized straight to fp8 by the DVE relu (relu commutes with positive
scale); W2 host-scaled by 64 so h2 (fp16) carries scale 2048; W3p
host-scaled by 2048/64 so the final tanh uses a 1/64 input scale constant.
End-to-end rel err ~9e-3 vs the 2e-2 gate (bias terms dominate the output
norm, diluting fp8 quantization noise on the data path).

Schedule (from perfetto-trace iteration):
  - weights live in DRAM pre-transposed with the SBUF partition dim
    outermost, so each weight matrix arrives in 2-3 large DMAs (a small
    transfer pays ~0.5us fixed overhead; singleton per-tile DMAs starved
    sweep-0 L1);
  - sweep-0 sigmoids are emitted BEFORE the weight dma_starts: the sync
    engine batches DMA-completion waits by program order, and emitting them
    after made the first sigmoid wait on five weight-block DMAs (~3.5us);
  - sigmoids for chunk n+1 are emitted during sweep n (ScalarE is in-order:
    they must sit AHEAD of tanh(n), which waits on the PE sel-matmul --
    otherwise every sweep seam idles PE ~2us and the idle re-throttles the
    PE clock gate to half rate for another 3.4us);
  - the layer-3 tail of sweep n (last quad, psum->fp16 copy, selection
    matmul, tanh, out-DMA) is deferred into the first two L1 groups of
    sweep n+1 so the DVE copy never blocks PE;
  - earlier L3 quads run inline, lagged two m-tiles behind their h2 relus;
  - PE warmup matmuls on an uninitialized tile (no producers -> start right
    after the engine preamble) open the HAM clock gate during the DMA
    window; a separate 1-element dummy activation preloads the ScalarE
    act table without making the warmup depend on ScalarE.
"""

import numpy as np
import ml_dtypes

import concourse.mybir as mybir
import concourse.tile as tile
from concourse import bacc
from concourse.bass_utils import run_bass_kernel_spmd

# Problem dims (hardcoded per harness contract)
B = 16384
D = 128
ENC_K = 10
ACT_DIM = 32
DEC_K = 10
H0 = 2048
H1 = 2048
NCORES = 8
BL = B // NCORES  # 2048 samples per core
NT = 512          # moving-dim tile (one PSUM bank of fp32)
NSUB = BL // NT   # 4
M1 = H0 // 128    # 16 m-tiles for layer 1
K1 = ENC_K        # 10 k-tiles for layer 1 (permuted encoder)
Q1 = K1 // 2      # 5 DoubleRow k-pairs
M2 = H1 // 128    # 16
K2 = H0 // 128    # 16
Q2 = K2 // 2      # 8 DoubleRow k-pairs
K3 = H1 // 128    # 16

F8 = mybir.dt.float8e4
F16 = mybir.dt.float16
F32 = mybir.dt.float32

# fp8 scale plumbing (all powers of 2; folded host-side into weights/biases)
SW1 = 32.0            # W1 and h1 fp8 scale
SW2 = 64.0            # W2 fp8 scale
SH2 = SW1 * SW2       # h2 (fp16) scale = 2048
SW3 = 32.0            # W3p divided by this -> psum3 scale SH2/SW3 = 64
TANH_SCALE = SW3 / SH2  # 1/64 input scale on the final tanh

DR = mybir.MatmulPerfMode.DoubleRow

_cached = {}


def _build_program():
    if "nc" in _cached:
        return _cached["nc"]

    nc = bacc.Bacc("TRN2", target_bir_lowering=False, debug=False,
                   num_devices=NCORES)

    stateT = nc.dram_tensor("stateT", [D, BL], F32, kind="ExternalInput")
    # weight layouts: [j, m, k, p]; j = within-tile contraction (SBUF
    # partition dim, outermost so one DMA descriptor per partition),
    # k = k-tile, p = output partition (lhsT free dim)
    w1t = nc.dram_tensor("w1t", [128, M1, K1, 128], F8, kind="ExternalInput")
    w2t = nc.dram_tensor("w2t", [128, M2, K2, 128], F8, kind="ExternalInput")
    w3t = nc.dram_tensor("w3t", [128, K3, ACT_DIM], F16, kind="ExternalInput")
    # scalars layout (per partition p): [0:10] enc_scale, [10:20] enc_bias,
    # [20:36] 32*b1, [36:52] 2048*b2, [52] b3p (partitions 0..31)
    scal = nc.dram_tensor("scal", [128, 53], F32, kind="ExternalInput")
    selt = nc.dram_tensor("selt", [128, ACT_DIM], F16, kind="ExternalInput")
    out = nc.dram_tensor("out", [ACT_DIM, BL], F32, kind="ExternalOutput")

    ADD = mybir.AluOpType.add
    MAX = mybir.AluOpType.max

    with tile.TileContext(nc) as tc:
        with (
            tc.tile_pool(name="consts", bufs=1) as consts,
            tc.tile_pool(name="acts", bufs=1) as acts,
            tc.tile_pool(name="h2p", bufs=18) as h2p,
            tc.tile_pool(name="w1p", bufs=1) as w1p,
            tc.tile_pool(name="w2p", bufs=1) as w2p,
            tc.tile_pool(name="outp", bufs=2) as outp,
            tc.tile_pool(name="psum", bufs=1, space="PSUM") as psum_pool,
        ):
            # state chunk 0 + scalars on the ACT DGE queue, separate from
            # the weight transfers: the scheduler coalesces DMA-completion
            # waits per queue, and sharing a queue with W1 made the first
            # sigmoid wait for megabytes of weights. A tiny GPSIMD read
            # after `sc` forces an exact sync boundary.
            st = acts.tile([D, BL], F32, tag="state")
            nc.scalar.dma_start(out=st[:, 0:NT], in_=stateT[:, 0:NT])
            sc = consts.tile([128, 53], F32)
            nc.scalar.dma_start(out=sc, in_=scal[:, :])

            # act-table trigger: tiny Sigmoid+Tanh on a dummy tile preloads
            # the ScalarE activation table during the preamble, without
            # making anything else depend on it
            dum = consts.tile([1, 4], F32, tag="dummy")
            nc.scalar.activation(dum[0:1, 0:1], dum[0:1, 0:1],
                                 mybir.ActivationFunctionType.Sigmoid)
            nc.scalar.activation(dum[0:1, 1:2], dum[0:1, 1:2],
                                 mybir.ActivationFunctionType.Tanh)
            nc.scalar.activation(dum[0:1, 2:3], dum[0:1, 2:3],
                                 mybir.ActivationFunctionType.Relu)

            w1all = w1p.tile([128, M1, K1, 128], F8, tag="w1")
            w2all = w2p.tile([128, M2, K2, 128], F8, tag="w2")
            w3sb = consts.tile([128, K3, ACT_DIM], F16, tag="w3")
            sel_sb = consts.tile([128, ACT_DIM], F16, tag="sel")

            # Persistent PSUM accumulators, rotated manually. Banks are
            # zeroed by DVE several groups before reuse, and matmul groups
            # run WITHOUT start=True: the group-start bank-clear blocks the
            # LDWEIGHTS pull-ahead and costs ~100ns per group.
            NPS = 5
            ps_tiles = [psum_pool.tile([128, NT], F32, tag=f"ps{i}",
                                       name=f"ps{i}")
                        for i in range(NPS)]
            ps_idx = [0]

            def next_ps():
                t = ps_tiles[ps_idx[0] % NPS]
                ps_idx[0] += 1
                return t

            # layer-3 col-packed accumulators (full bank, 4 col-groups) and
            # the [32, NT] reduce target
            p3_tiles = [psum_pool.tile([128, NT], F32, tag=f"p3_{i}",
                                       name=f"p3t_{i}")
                        for i in range(2)]
            psr = psum_pool.tile([ACT_DIM, NT], F32, tag="psr", name="psr")

            # ---- PE warmup: dummy matmuls on an uninitialized tile (no
            # producers, so they dispatch the moment the preamble ends) keep
            # PE busy through the DMA window and open the HAM clock gate.
            # Results are garbage and discarded (psr is DVE-zeroed below).
            wz = consts.tile([128, NT], F16, tag="warmzero")
            # 1-element DVE write so the tile gets allocated; the rest stays
            # uninitialized garbage (results are discarded)
            nc.vector.memset(wz[0:1, 0:1], 0.0)
            NWARM = 6
            for w in range(NWARM):
                nc.tensor.matmul(
                    psr, wz[:, :ACT_DIM], wz,
                    start=(w == 0), stop=(w == NWARM - 1),
                    skip_group_check=True)

            # k-PAIR tiles (DoubleRow consumes two adjacent k-subtiles per
            # matmul); per-pair tiles keep dependency tracking fine-grained
            spine = [acts.tile([128, 2, BL], F8, tag=f"spine{q}",
                               name=f"spine{q}")
                     for q in range(Q1)]
            h1 = [acts.tile([128, 2, BL], F8, tag=f"h1_{p}", name=f"h1_{p}")
                  for p in range(Q2)]

            def emit_sig(n):
                ns = slice(n * NT, (n + 1) * NT)
                for k in range(K1):
                    nc.scalar.activation(
                        spine[k // 2][:, k % 2, ns], st[:, ns],
                        mybir.ActivationFunctionType.Sigmoid,
                        bias=sc[:, 10 + k:11 + k], scale=sc[:, k:k + 1])

            # sweep-0 sigmoids at scheduler priority 0: their sync-engine
            # wait ops must sit at the SP queue head, before anything tied
            # to the weight DMAs, so they gate on the state-chunk-0 + scal
            # transfers only (measured otherwise: first sigmoid at 18us,
            # PE idle 11us). This also pushes the W2 triggers behind the
            # sigmoids on ScalarE, giving W1 the full HBM bandwidth first.
            with tc.high_priority():
                emit_sig(0)

            for t in ps_tiles + p3_tiles:
                nc.vector.memset(t, 0.0)

            # DMA-completion waits are DRAIN waits: a consumer waits for
            # every transfer whose trigger was scheduled before the
            # consumer's wait op on that queue. So each trigger is pinned
            # (tile_wait_until, sim-us) to land just AFTER the waits of the
            # consumers that must not drain it, and just before its own
            # consumer needs the data. W1 chunks ride the SP queue; state
            # chunk 0 + scalars + W2 bulk ride the ACT queue (whose trigger
            # ops also cost ~0.7us of ScalarE each, so W2 naturally queues
            # behind the sweep-0 sigmoids).
            nc.sync.dma_start(out=w1all[:, 0:NPS], in_=w1t[:, 0:NPS])
            with tc.tile_wait_until(0.0123):
                nc.sync.dma_start(out=w1all[:, NPS:10], in_=w1t[:, NPS:10])
            with tc.tile_wait_until(0.017):
                nc.sync.dma_start(out=w1all[:, 10:M1], in_=w1t[:, 10:M1])
            with tc.tile_wait_until(0.023):
                nc.sync.dma_start(out=st[:, NT:BL], in_=stateT[:, NT:BL])
                nc.sync.dma_start(out=w3sb, in_=w3t[:, :, :])
                nc.sync.dma_start(out=sel_sb, in_=selt[:, :])
            with tc.tile_wait_until(0.012):
                nc.scalar.dma_start(out=w2all[:, 0:8], in_=w2t[:, 0:8])
            with tc.tile_wait_until(0.030):
                nc.scalar.dma_start(out=w2all[:, 8:M2], in_=w2t[:, 8:M2])

            def emit_quad(q, p3, h2s, nj=4):
                # one col-packed quad: up to 4 concurrent 32-wide matmuls
                # (priority 0 so the scheduler pops them adjacently the
                # moment their h2 inputs are ready)
                with tc.high_priority():
                    for j in range(nj):
                        nc.tensor.matmul(
                            p3[32 * j:32 * (j + 1), :], w3sb[:, 4 * q + j, :],
                            h2s[j], start=False, stop=False,
                            skip_group_check=True, tile_position=(0, 32 * j))

            def emit_fin(n, p3):
                # cross-col-group reduce + tanh + store for sweep n. The s3
                # copy AND the p3 re-zeroing run on ScalarE (not DVE), and
                # psr needs no zeroing (sel starts its own group): the tail
                # touches only ScalarE+PE+DMA, so it can never wedge the
                # DVE relu/memset stream that feeds the PSUM rotation.
                s3 = outp.tile([128, NT], F16, tag="s3", name=f"s3_{n}")
                nc.scalar.activation(s3, p3,
                                     mybir.ActivationFunctionType.Copy)
                nc.scalar.mul(p3, p3, 0.0)
                with tc.high_priority():
                    nc.tensor.matmul(psr, sel_sb, s3, start=True,
                                     stop=True, skip_group_check=True)
                ot = outp.tile([ACT_DIM, NT], F32, tag="ot", name=f"ot_{n}")
                nc.scalar.activation(
                    ot, psr, mybir.ActivationFunctionType.Tanh,
                    bias=sc[:ACT_DIM, 52:53], scale=TANH_SCALE)
                nc.sync.dma_start(out=out[:, n * NT:(n + 1) * NT], in_=ot)

            def l1_mms(ps, m, ns):
                for q in range(Q1):
                    nc.tensor.matmul(
                        ps, w1all[:, m, 2 * q:2 * q + 2, :],
                        spine[q][:, :, ns],
                        start=False, stop=False, skip_group_check=True,
                        perf_mode=DR)

            # deferred layer-3 tail of the previous sweep: (kind, args)
            tail = []

            # ---- fully interleaved per-column-chunk sweeps:
            # sigmoid(n+1) -> L1 m-sweep(n) [+ prev sweep's L3 tail]
            # -> L2 m-sweep(n) [+ inline lagged quads]
            for n in range(NSUB):
                ns = slice(n * NT, (n + 1) * NT)

                m_start = 0
                if n == 0:
                    # k-pair-striped cohort over all 5 banks: each pair-wave
                    # only needs two freshly produced spine slices, so real
                    # L1 work runs during the ScalarE sigmoid ramp.
                    m_start = NPS
                    cohort = [next_ps() for _ in range(NPS)]
                    for q in range(Q1):
                        for m in range(NPS):
                            nc.tensor.matmul(
                                cohort[m], w1all[:, m, 2 * q:2 * q + 2, :],
                                spine[q][:, :, ns],
                                start=False, stop=False,
                                skip_group_check=True, perf_mode=DR)
                    for m in range(NPS):
                        nc.vector.tensor_scalar(
                            h1[m // 2][:, m % 2, ns], cohort[m],
                            sc[:, 20 + m:21 + m], 0.0, ADD, MAX)
                        nc.vector.memset(cohort[m], 0.0)

                for idx, m in enumerate(range(m_start, M1)):
                    if n == 0 and m == m_start:
                        # p3_tiles[1] is idle until sweep 1's layer 3 — use
                        # it here so this group does not wait on the cohort's
                        # first relu+memset to release a rotation bank.
                        ps = p3_tiles[1]
                    else:
                        ps = next_ps()
                    l1_mms(ps, m, ns)
                    # relus alternate DVE/ScalarE: DVE alone (relu+memset
                    # ~1.23us) cannot keep up with the 1.08us L1 PE groups,
                    # and the rotation memsets must never starve
                    if m % 2 == 0:
                        nc.vector.tensor_scalar(
                            h1[m // 2][:, m % 2, ns], ps,
                            sc[:, 20 + m:21 + m], 0.0, ADD, MAX)
                    else:
                        nc.scalar.activation(
                            h1[m // 2][:, m % 2, ns], ps,
                            mybir.ActivationFunctionType.Relu,
                            bias=sc[:, 20 + m:21 + m])
                    nc.vector.memset(ps, 0.0)
                    # previous sweep's L3 tail, spread over the first two
                    # L1 groups (PE reaches the quad after ~1.1us of L1, by
                    # which time the last h2 relu is done; the sel matmul
                    # lands after the DVE s3 copy)
                    while tail and tail[0][0] <= idx:
                        _, fn, args = tail.pop(0)
                        fn(*args)

                if n + 1 < NSUB:
                    emit_sig(n + 1)

                p3 = p3_tiles[n % 2]
                h2s_sweep = []
                last = (n + 1 == NSUB)
                for m in range(M2):
                    if last and m == M2 - 1:
                        # last sweep: m12-14 go into col groups 0-2 as a
                        # partial quad and the s3 copy + sel reduce run
                        # DURING m15's matmul group, so only
                        # relu -> one m15-matmul -> tanh -> store remain
                        # on the critical tail after the last L2 group
                        emit_quad(3, p3, h2s_sweep[12:15], nj=3)
                        s3l = outp.tile([128, NT], F16, tag="s3",
                                        name="s3_last")
                        nc.scalar.activation(
                            s3l, p3, mybir.ActivationFunctionType.Copy)
                        with tc.high_priority():
                            nc.tensor.matmul(psr, sel_sb, s3l, start=True,
                                             stop=False,
                                             skip_group_check=True)
                    ps = next_ps()
                    for q in range(Q2):
                        nc.tensor.matmul(
                            ps, w2all[:, m, 2 * q:2 * q + 2, :],
                            h1[q][:, :, ns],
                            start=False, stop=False, skip_group_check=True,
                            perf_mode=DR)
                    h2m = h2p.tile([128, NT], F16, tag="h2")
                    if last and m == M2 - 1:
                        # split the tail-critical relu across DVE+ScalarE
                        HH = NT // 2
                        nc.vector.tensor_scalar(
                            h2m[:, 0:HH], ps[:, 0:HH],
                            sc[:, 36 + m:37 + m], 0.0, ADD, MAX)
                        nc.scalar.activation(
                            h2m[:, HH:NT], ps[:, HH:NT],
                            mybir.ActivationFunctionType.Relu,
                            bias=sc[:, 36 + m:37 + m])
                    else:
                        nc.vector.tensor_scalar(
                            h2m, ps, sc[:, 36 + m:37 + m], 0.0, ADD, MAX)
                    nc.vector.memset(ps, 0.0)
                    h2s_sweep.append(h2m)
                    # quads q0-q2 inline, lagged two m-tiles behind their
                    # relus; q3 is deferred into the next sweep's L1
                    if m in (5, 9, 13):
                        emit_quad((m - 5) // 4, p3, h2s_sweep[m - 5:m - 1])

                if not last:
                    tail = [(0, emit_quad, (3, p3, h2s_sweep[12:16])),
                            (1, emit_fin, (n, p3))]
                else:
                    nc.tensor.matmul(psr, w3sb[:, M2 - 1, :],
                                     h2s_sweep[M2 - 1], start=False,
                                     stop=True, skip_group_check=True)
                    ot = outp.tile([ACT_DIM, NT], F32, tag="ot", name="ot_3")
                    nc.scalar.activation(
                        ot, psr, mybir.ActivationFunctionType.Tanh,
                        bias=sc[:ACT_DIM, 52:53], scale=TANH_SCALE)
                    nc.sync.dma_start(out=out[:, n * NT:(n + 1) * NT],
                                      in_=ot)

    nc.compile()
    _cached["nc"] = nc
    return nc


def _q8(x):
    """fp32 -> TRN e4m3 bytes (clip to +-240; bit-identical to OCP there)."""
    return np.clip(x, -240.0, 240.0).astype(ml_dtypes.float8_e4m3fn)


def _prep_inputs(state, mean_enc, std_enc, W1, b1, W2, b2, W3, b3, Wd, bd):
    f32 = np.float32
    state = np.asarray(state, f32)
    mean_enc = np.asarray(mean_enc, f32)
    std_enc = np.asarray(std_enc, f32)
    W1 = np.asarray(W1, f32)
    b1 = np.asarray(b1, f32)
    W2 = np.asarray(W2, f32)
    b2 = np.asarray(b2, f32)
    W3 = np.asarray(W3, f32)
    b3 = np.asarray(b3, f32)
    Wd = np.asarray(Wd, f32)
    bd = np.asarray(bd, f32)

    # Fold decoder grouped conv into layer 3
    wd = Wd[:, 0, :]                                   # [32, 10]
    W3p = np.einsum("ak,akh->ah", wd, W3.reshape(ACT_DIM, DEC_K, H1))
    b3p = (b3.reshape(ACT_DIM, DEC_K) * wd).sum(1) + bd  # [32]

    # Permute encoder contraction: j' = k*128 + d
    W1p = W1.reshape(H0, D, ENC_K).transpose(0, 2, 1).reshape(H0, D * ENC_K)

    # Pre-tiled weight layouts: [j, m, k, p] (partition dim outermost so
    # each weight matrix is a handful of large contiguous DMAs)
    w1t = np.ascontiguousarray(
        _q8((W1p * SW1).reshape(M1, 128, K1, 128).transpose(3, 0, 2, 1)))
    w2t = np.ascontiguousarray(
        _q8((W2 * SW2).reshape(M2, 128, K2, 128).transpose(3, 0, 2, 1)))
    w3t = np.ascontiguousarray(
        (W3p / SW3).reshape(ACT_DIM, K3, 128).transpose(2, 1, 0)
        .astype(np.float16))

    scal = np.zeros((128, 53), f32)
    scal[:, 0:10] = 1.0 / std_enc[0]                   # enc scale [128, 10]
    scal[:, 10:20] = -mean_enc[0] / std_enc[0]         # enc bias
    scal[:, 20:36] = SW1 * b1.reshape(M1, 128).T
    scal[:, 36:52] = SH2 * b2.reshape(M2, 128).T
    scal[:ACT_DIM, 52] = b3p
    scal = np.ascontiguousarray(scal)

    sel = np.tile(np.eye(ACT_DIM, dtype=np.float16), (4, 1))

    in_maps = []
    for c in range(NCORES):
        shard = np.ascontiguousarray(state[c * BL:(c + 1) * BL].T)  # [128, BL]
        in_maps.append({
            "stateT": shard, "w1t": w1t, "w2t": w2t, "w3t": w3t,
            "scal": scal, "selt": sel,
        })
    return in_maps


def kernel(**inputs):
    nc = _build_program()
    in_maps = _prep_inputs(**inputs)
    res = run_bass_kernel_spmd(nc, in_maps, core_ids=list(range(NCORES)))
    out = np.concatenate(
        [res.results[c]["out"].T for c in range(NCORES)], axis=0)
    return np.ascontiguousarray(out.astype(np.float32))


if __name__ == "__main__":
    rng = np.random.default_rng(0)
    state = rng.standard_normal((B, D), dtype=np.float32)
    mean = np.broadcast_to(
        np.linspace(-3, 3, ENC_K, dtype=np.float32), (1, D, ENC_K)).copy()
    std = np.full((1, D, ENC_K), 0.3872983346207417, np.float32)

    def lin(fan_in, fan_out):
        bound = 1 / np.sqrt(fan_in)
        return (rng.uniform(-bound, bound, (fan_out, fan_in)).astype(np.float32),
                rng.uniform(-bound, bound, fan_out).astype(np.float32))

    W1, b1 = lin(D * ENC_K, H0)
    W2, b2 = lin(H0, H1)
    W3, b3 = lin(H1, ACT_DIM * DEC_K)
    Wd = rng.uniform(-0.3, 0.3, (ACT_DIM, 1, DEC_K)).astype(np.float32)
    bd = rng.uniform(-0.3, 0.3, ACT_DIM).astype(np.float32)

    outp = kernel(state=state, mean_enc=mean, std_enc=std, W1=W1, b1=b1,
                  W2=W2, b2=b2, W3=W3, b3=b3, Wd=Wd, bd=bd)

    # numpy reference
    spine = 1 / (1 + np.exp(-(state[:, :, None] - mean) / std))
    a = np.maximum(spine.reshape(B, -1) @ W1.T + b1, 0)
    a = np.maximum(a @ W2.T + b2, 0)
    a = a @ W3.T + b3
    raw = np.einsum("bak,ak->ba", a.reshape(B, ACT_DIM, DEC_K), Wd[:, 0]) + bd
    ref = np.tanh(raw)
    rel = np.linalg.norm(outp - ref) / np.linalg.norm(ref)
    print("rel err:", rel, "max abs diff:", np.abs(outp - ref).max())
